# revision 22
# baseline (speedup 1.0000x reference)
"""GPSA (gated positional self-attention) Bass kernel for Trainium2, v2.

Sharding: 8 cores = 4 batches x 2 query-halves. Each core: full keys
(N=1024), 512 queries, all 8 heads.

Per-head math (core = batch b, query half r):
  patch: e2 = exp(scale * k^T q) computed key-major [keys, q] (no max
         needed, logits are tiny). PV uses a 33-column V-block
         [(1-g)*v_h | ones] so column 32 of the accumulator carries S2.
  pos:   e1 = exp(w3_h * d - m1) query-major [q, keys]; m1 = w3*dmax or
         w3*dmin (exact row max of w3*d). The per-key bias term
         exp(bh - bmax + DELTA) is folded into the pos V-block host-side:
         [g*expbh*v_h | expbh], so column 32 carries U1 = sum e1*expbh.
         e1 is transposed to key-major via DMA xbar transpose (bf16,
         3D out AP -> keys land interleaved k=8p+j; vp rows are host-
         permuted to match).
  combine (per query chunk qi): PE-transpose both 33-row accumulators to
         [q, 33]; reciprocal of column 32 gives per-partition 1/S2, 1/U1;
         two DVE ops apply both normalizations and the add. Final
         transpose back to [C, q] for the output projection.

All matmuls bf16 except the distance Gram trick (fp32 for cancellation).
Single act-table switch: all sqrts emitted before all exps.
"""
import sys
import numpy as np

sys.path.insert(0, "/opt/trn_rl_repo")

import concourse.bass as bass  # noqa: E402
import concourse.tile as tile  # noqa: E402
from concourse import bacc, mybir  # noqa: E402
from concourse.bass_utils import run_bass_kernel_spmd  # noqa: E402

LAST_RESULTS = None  # BassKernelResults of the most recent kernel() call

B, N, C, H = 4, 1024, 256, 8
HD = C // H           # 32
NQ = N // 2           # 512 queries per core
NKC = N // 128        # 8 key chunks
NQC = NQ // 128       # 4 query chunks
SCALE = HD ** -0.5
DELTA = 50.0          # pos bias shift: expbh = exp(bh - bmax + DELTA)
FP = mybir.dt.float32
BF = mybir.dt.bfloat16
AF = mybir.ActivationFunctionType
ALU = mybir.AluOpType
AX = mybir.AxisListType


def _build(w3, gh):
    """w3: Wpos[:,3] (8 floats); gh: sigmoid(gating) (8 floats) as immediates."""
    nc = bacc.Bacc("TRN2", target_bir_lowering=False)

    # bf16 inputs arrive as two [128, BLOBW] blobs (one DMA each):
    # [xT (1024) | xTq (512) | wq (256) | wk (256) | wp (256) | id128 (128)]
    BLOBW = N + NQ + 3 * C + 128
    d_blob = [nc.dram_tensor(f"blob{i}", (128, BLOBW), BF, kind="ExternalInput")
              for i in range(2)]
    d_va = nc.dram_tensor("va", (128, NKC * 272), BF, kind="ExternalInput")
    d_vp = nc.dram_tensor("vp", (128, NKC * 272), BF, kind="ExternalInput")
    d_cT = nc.dram_tensor("cT", (4, N), FP, kind="ExternalInput")      # row 3 = sq_k
    d_cm2 = nc.dram_tensor("cm2q", (4, NQ), FP, kind="ExternalInput")  # row 3 = 1.0
    d_sqq = nc.dram_tensor("sqq4", (128, NQC), FP, kind="ExternalInput")
    d_bp = nc.dram_tensor("bproj", (128, 2), FP, kind="ExternalInput")
    d_y = nc.dram_tensor("yT", (C, NQ), FP, kind="ExternalOutput")

    with tile.TileContext(nc) as tc:
        with (
            tc.tile_pool(name="const", bufs=1) as cpool,
            tc.tile_pool(name="work", bufs=3) as wpool,
            tc.tile_pool(name="ebuf", bufs=4) as epool,
            tc.tile_pool(name="etbuf", bufs=2) as etpool,
            tc.tile_pool(name="psw", bufs=2, space=bass.MemorySpace.PSUM) as pp,
            tc.tile_pool(name="psacc", bufs=2, space=bass.MemorySpace.PSUM) as pacc,
            tc.tile_pool(name="psT", bufs=2, space=bass.MemorySpace.PSUM) as pT,
            tc.tile_pool(name="psotp", bufs=1, space=bass.MemorySpace.PSUM) as potp,
        ):
            # ---- constants / inputs ----
            # small fp32 tensors first (pos prep depends only on these),
            # issued on the scalar queue so they don't serialize behind the
            # big bf16 blobs on sync.
            cT = cpool.tile([4, N], FP, tag="cT")
            cm2 = cpool.tile([4, NQ], FP, tag="cm2")
            sqq4 = cpool.tile([128, NQC], FP, tag="sqq4")
            bp2 = cpool.tile([128, 2], FP, tag="bp2")
            nc.scalar.dma_start(cT[:], d_cT[:])
            nc.scalar.dma_start(cm2[:], d_cm2[:])
            nc.scalar.dma_start(sqq4[:], d_sqq[:])
            nc.scalar.dma_start(bp2[:], d_bp[:])
            blob = [cpool.tile([128, BLOBW], BF, tag=f"blob{i}", name=f"blob{i}")
                    for i in range(2)]
            nc.sync.dma_start(blob[0][:], d_blob[0][:])
            nc.sync.dma_start(blob[1][:], d_blob[1][:])
            va = cpool.tile([128, NKC * 272], BF, tag="va")
            vp = cpool.tile([128, NKC * 272], BF, tag="vp")
            nc.sync.dma_start(va[:], d_va[:])
            nc.sync.dma_start(vp[:], d_vp[:])
            o_xT, o_xTq, o_wq, o_wk, o_wp, o_id = (
                0, N, N + NQ, N + NQ + C, N + NQ + 2 * C, N + NQ + 3 * C)
            xT = [blob[i][:, o_xT:o_xT + N] for i in range(2)]
            xTq = [blob[i][:, o_xTq:o_xTq + NQ] for i in range(2)]
            wq = [blob[i][:, o_wq:o_wq + C] for i in range(2)]
            wk = [blob[i][:, o_wk:o_wk + C] for i in range(2)]
            wp = [blob[i][:, o_wp:o_wp + C] for i in range(2)]
            id128 = blob[0][:, o_id:o_id + 128]
            sqq = [sqq4[:, qi:qi + 1] for qi in range(NQC)]
            bp = [bp2[:, i:i + 1] for i in range(2)]

            # ---- projections qT [C, NQ], kT [C, N] (bf16) ----
            # rows 96:128 (heads 3, 7) are mirrored to base-0 tiles: matmul
            # operands at base_partition 96 are not supported.
            q_sb = [cpool.tile([128, NQ], BF, tag=f"q{i}", name=f"qsb{i}") for i in range(2)]
            k_sb = [cpool.tile([128, N], BF, tag=f"k{i}", name=f"ksb{i}") for i in range(2)]
            q_e = [cpool.tile([32, NQ], BF, tag=f"qe{i}", name=f"qe{i}") for i in range(2)]
            k_e = [cpool.tile([32, N], BF, tag=f"ke{i}", name=f"ke{i}") for i in range(2)]
            for co in range(2):
                cs = slice(co * 128, (co + 1) * 128)
                ps = pp.tile([128, NQ], FP, tag="ps")
                for ci in range(2):
                    nc.tensor.matmul(ps[:], wq[ci][:, cs], xTq[ci][:],
                                     start=(ci == 0), stop=(ci == 1))
                nc.vector.tensor_copy(q_sb[co][:], ps[:])
                nc.vector.tensor_copy(q_e[co][:], ps[96:128, :])
                for half in range(2):
                    hs = slice(half * 512, (half + 1) * 512)
                    ps2 = pp.tile([128, 512], FP, tag="ps")
                    for ci in range(2):
                        nc.tensor.matmul(ps2[:], wk[ci][:, cs], xT[ci][:, hs],
                                         start=(ci == 0), stop=(ci == 1))
                    nc.vector.tensor_copy(k_sb[co][:, hs], ps2[:])
                    nc.vector.tensor_copy(k_e[co][:, hs], ps2[96:128, :])

            # ---- patch attention, key-major; o2 row 32 = S2 ----
            # o_sb tiles are 34 rows (row 33 zero from the padded V block) so
            # the combine transposes write 4-byte-aligned bf16 PSUM blocks.
            o2_sb = [cpool.tile([34, NQ], BF, tag=f"o2_{h}", name=f"o2sb{h}") for h in range(H)]

            def emit_patch(h):
                hc, j3 = h // 4, h % 4
                hr = j3 * 32
                q_h = q_e[hc][:] if j3 == 3 else q_sb[hc][hr:hr + 32, :]
                o2 = pacc.tile([34, NQ], FP, tag="acc", name=f"o2acc{h}")
                for kc in range(NKC):
                    ks = slice(kc * 128, (kc + 1) * 128)
                    k_h = k_e[hc][:, ks] if j3 == 3 else k_sb[hc][hr:hr + 32, ks]
                    s2 = pp.tile([128, NQ], FP, tag="ps", name=f"s2_{h}_{kc}")
                    nc.tensor.matmul(s2[:], k_h, q_h, start=True, stop=True)
                    e2 = wpool.tile([128, NQ], BF, tag="e2", name=f"e2_{h}_{kc}")
                    nc.scalar.activation(e2[:], s2[:], AF.Exp, scale=SCALE)
                    nc.tensor.matmul(o2[:], va[:, kc * 272 + h * 34:kc * 272 + (h + 1) * 34],
                                     e2[:], start=(kc == 0), stop=(kc == NKC - 1))
                nc.vector.tensor_copy(o2_sb[h][:], o2[:])

            # ---- pos distance prep: d = sqrt(max(0, cm2.cT + sq_q)) ----
            # cT row 3 carries sq_k and cm2 row 3 is 1.0, so a single fp32
            # matmul yields -2c.c + sq_k; sq_q rides the DVE op as a
            # per-partition scalar, fused with the >=0 clamp.
            d_q = [cpool.tile([128, N], FP, tag=f"d{qi}", name=f"d{qi}") for qi in range(NQC)]
            dmax = [cpool.tile([128, 1], FP, tag=f"dmax{qi}", name=f"dmax{qi}") for qi in range(NQC)]
            dmin = [cpool.tile([128, 1], FP, tag=f"dmin{qi}", name=f"dmin{qi}") for qi in range(NQC)]

            def emit_prep(qi):
                qs = slice(qi * 128, (qi + 1) * 128)
                for half in range(2):
                    hs = slice(half * 512, (half + 1) * 512)
                    dps = pp.tile([128, 512], FP, tag="ps", name=f"dps{qi}_{half}")
                    nc.tensor.matmul(dps[:], cm2[:, qs], cT[:, hs],
                                     start=True, stop=True)
                    d2 = wpool.tile([128, 512], FP, tag="d2", name=f"d2_{qi}_{half}")
                    nc.vector.tensor_scalar(d2[:], dps[:], sqq[qi], 0.0,
                                            op0=ALU.add, op1=ALU.max)
                    nc.scalar.sqrt(d_q[qi][:, hs], d2[:])
                nc.vector.tensor_reduce(dmax[qi][:], d_q[qi][:], AX.X, ALU.max)
                nc.vector.tensor_reduce(dmin[qi][:], d_q[qi][:], AX.X, ALU.min)

            # emission order keeps the PE dense from the start (HAM warm):
            # two patch heads, then the pos prep (whose sqrts sit between
            # exp blocks on the scalar queue: 2 act-table switches total),
            # then the remaining patch heads.
            emit_patch(0)
            emit_patch(1)
            for qi in range(NQC):
                emit_prep(qi)
            for h in range(2, H):
                emit_patch(h)

            # ---- pos attention: e1 query-major -> DMA xbar transpose ----
            o1_sb = [cpool.tile([34, NQ], BF, tag=f"o1_{h}", name=f"o1sb{h}") for h in range(H)]
            for h in range(H):
                dext = dmax if w3[h] > 0 else dmin
                # e1T[p, j, qi, q] = e1[qi*128+q, 8p+j]
                e1T = etpool.tile([128, NKC, NQC, 128], BF, tag="e1T")
                for qi in range(NQC):
                    negm = wpool.tile([128, 1], FP, tag="negm")
                    nc.vector.tensor_scalar_mul(negm[:], dext[qi][:], -float(w3[h]))
                    e1 = epool.tile([128, N], BF, tag="e1")
                    nc.scalar.activation(e1[:], d_q[qi][:], AF.Exp,
                                         bias=negm[:], scale=float(w3[h]))
                    nc.sync.dma_start(e1T[:, :, qi, :], e1[:], transpose=True)
                o1 = pacc.tile([34, NQ], FP, tag="acc")
                for j in range(NKC):
                    nc.tensor.matmul(o1[:], vp[:, j * 272 + h * 34:j * 272 + (h + 1) * 34],
                                     e1T[:, j, :, :], start=(j == 0), stop=(j == NKC - 1))
                nc.vector.tensor_copy(o1_sb[h][:], o1[:])

            # ---- combine per query chunk: transpose accs, normalize, add ----
            oT = [cpool.tile([128, NQ], BF, tag=f"oT{i}", name=f"oTsb{i}") for i in range(2)]
            for qi in range(NQC):
                qs = slice(qi * 128, (qi + 1) * 128)
                T2 = pT.tile([128, 272], BF, tag="T")
                T1 = pT.tile([128, 272], BF, tag="T")
                for h in range(H):
                    nc.tensor.transpose(T2[:, h * 34:(h + 1) * 34],
                                        o2_sb[h][:, qs], id128[0:34, 0:34])
                    nc.tensor.transpose(T1[:, h * 34:(h + 1) * 34],
                                        o1_sb[h][:, qs], id128[0:34, 0:34])
                r2 = wpool.tile([128, H], FP, tag="r2")
                r1 = wpool.tile([128, H], FP, tag="r1")
                nc.vector.reciprocal(r2[:], T2[:, 32::34])
                nc.vector.reciprocal(r1[:], T1[:, 32::34])
                oq = wpool.tile([128, C], BF, tag="oq")
                for h in range(H):
                    t = wpool.tile([128, HD], FP, tag="t")
                    nc.vector.tensor_scalar_mul(t[:], T2[:, h * 34:h * 34 + 32],
                                                r2[:, h:h + 1])
                    nc.vector.scalar_tensor_tensor(
                        oq[:, h * 32:(h + 1) * 32], T1[:, h * 34:h * 34 + 32],
                        r1[:, h:h + 1], t[:], op0=ALU.mult, op1=ALU.add)
                for ci in range(2):
                    otp = potp.tile([128, 128], BF, tag="otp")
                    nc.tensor.transpose(otp[:], oq[:, ci * 128:(ci + 1) * 128],
                                        id128[:])
                    nc.vector.tensor_copy(oT[ci][:, qs], otp[:])

            # ---- final projection yT = Wproj @ OT + bproj ----
            for co in range(2):
                cs = slice(co * 128, (co + 1) * 128)
                yp = pp.tile([128, NQ], FP, tag="ps")
                for ci in range(2):
                    nc.tensor.matmul(yp[:], wp[ci][:, cs], oT[ci][:],
                                     start=(ci == 0), stop=(ci == 1))
                y = wpool.tile([128, NQ], FP, tag="y")
                nc.scalar.activation(y[:], yp[:], AF.Identity, bias=bp[co][:])
                nc.sync.dma_start(d_y[cs, :], y[:])

    nc.compile()
    return nc


def kernel(x, voxel_coord, Wqk, Wv, Wpos, bpos, Wproj, bproj, gating):
    x = np.asarray(x, np.float32)
    c = np.asarray(voxel_coord, np.float32)
    Wqk = np.asarray(Wqk, np.float32)
    Wv = np.asarray(Wv, np.float32)
    Wpos = np.asarray(Wpos, np.float32)
    bpos = np.asarray(bpos, np.float32)
    Wproj = np.asarray(Wproj, np.float32)
    bproj = np.asarray(bproj, np.float32)
    gating = np.asarray(gating, np.float32)

    import ml_dtypes
    bf16 = ml_dtypes.bfloat16

    w3 = [float(v) for v in Wpos[:, 3]]
    gh = [float(v) for v in 1.0 / (1.0 + np.exp(-gating))]
    nc = _build(w3, gh)

    WqT = np.ascontiguousarray(Wqk[:C].T).astype(bf16)
    WkT = np.ascontiguousarray(Wqk[C:].T).astype(bf16)
    WprojT = np.ascontiguousarray(Wproj.T).astype(bf16)
    bp2 = np.ascontiguousarray(bproj.reshape(2, 128).T)
    id128 = np.eye(128, dtype=np.float32)

    # v = x @ Wv.T; Wv is identity in this model, skip the matmul then.
    if np.array_equal(Wv, np.eye(C, dtype=np.float32)):
        v_full = x
    else:
        v_full = x @ Wv.T

    c = c - c.mean(axis=1, keepdims=True)  # shrink |c|^2 for Gram precision

    in_maps = []
    for core in range(8):
        b, r = core // 2, core % 2
        qs = slice(r * NQ, (r + 1) * NQ)
        xTb = np.ascontiguousarray(x[b].T)                      # (C, N) f32
        sq = np.sum(c[b] * c[b], axis=1).astype(np.float32)     # (N,)
        cTb = np.empty((4, N), np.float32)
        cTb[:3] = c[b].T
        cTb[3] = sq                                             # sq_k via Gram row
        cm2 = np.empty((4, NQ), np.float32)
        cm2[:3] = -2.0 * cTb[:3, qs]
        cm2[3] = 1.0
        bh = -(Wpos[:, :3] @ c[b].T) + bpos[:, None]            # (H, N)
        expbh = np.exp(bh - bh.max(axis=1, keepdims=True) + DELTA)  # (H, N)

        # patch V block: [(1-g)*v_h | ones] per (kc, h), native key order
        va = np.zeros((128, NKC * 272), np.float32)
        # pos V block: [g*expbh*v_h | expbh], interleaved key order k=8p+j
        vp = np.zeros((128, NKC * 272), np.float32)
        vb = v_full[b]                                          # (N, C)
        for kc in range(NKC):
            keys = np.arange(kc * 128, (kc + 1) * 128)
            for h in range(H):
                col = kc * 272 + h * 34
                va[:, col:col + 32] = (1.0 - gh[h]) * vb[keys, h * 32:(h + 1) * 32]
                va[:, col + 32] = 1.0
                vp[:, col:col + 32] = (gh[h] * expbh[h, keys, None]
                                       * vb[keys, h * 32:(h + 1) * 32])
                vp[:, col + 32] = expbh[h, keys]

        blobs = []
        for i in range(2):
            rs = slice(i * 128, (i + 1) * 128)
            idpart = id128 if i == 0 else np.zeros((128, 128), np.float32)
            blobs.append(np.concatenate([
                xTb[rs].astype(bf16), xTb[rs, qs].astype(bf16),
                WqT[rs], WkT[rs], WprojT[rs], idpart.astype(bf16),
            ], axis=1))
        in_maps.append({
            "blob0": np.ascontiguousarray(blobs[0]),
            "blob1": np.ascontiguousarray(blobs[1]),
            "va": va.astype(bf16), "vp": vp.astype(bf16),
            "cT": cTb, "cm2q": cm2,
            "sqq4": np.ascontiguousarray(sq[qs].reshape(NQC, 128).T),
            "bproj": bp2,
        })

    global LAST_RESULTS
    LAST_RESULTS = run_bass_kernel_spmd(nc, in_maps, list(range(8)))
    res = LAST_RESULTS.results
    out = np.empty((B, N, C), np.float32)
    for core in range(8):
        b, r = core // 2, core % 2
        out[b, r * NQ:(r + 1) * NQ, :] = res[core]["yT"].T
    return out


# revision 24
# speedup vs baseline: 1.3160x; 1.3160x over previous
"""GPSA (gated positional self-attention) Bass kernel for Trainium2, v2.

Sharding: 8 cores = 4 batches x 2 query-halves. Each core: full keys
(N=1024), 512 queries, all 8 heads.

Per-head math (core = batch b, query half r):
  patch: e2 = exp(scale * k^T q) computed key-major [keys, q] (no max
         needed, logits are tiny). PV uses a 33-column V-block
         [(1-g)*v_h | ones] so column 32 of the accumulator carries S2.
  pos:   e1 = exp(w3_h * d - m1) query-major [q, keys]; m1 = w3*dmax or
         w3*dmin (exact row max of w3*d). The per-key bias term
         exp(bh - bmax + DELTA) is folded into the pos V-block host-side:
         [g*expbh*v_h | expbh], so column 32 carries U1 = sum e1*expbh.
         e1 is transposed to key-major via DMA xbar transpose (bf16,
         3D out AP -> keys land interleaved k=8p+j; vp rows are host-
         permuted to match).
  combine (per query chunk qi): PE-transpose both 33-row accumulators to
         [q, 33]; reciprocal of column 32 gives per-partition 1/S2, 1/U1;
         two DVE ops apply both normalizations and the add. Final
         transpose back to [C, q] for the output projection.

All matmuls bf16 except the distance Gram trick (fp32 for cancellation).
Single act-table switch: all sqrts emitted before all exps.
"""
import sys
import numpy as np

sys.path.insert(0, "/opt/trn_rl_repo")

import concourse.bass as bass  # noqa: E402
import concourse.tile as tile  # noqa: E402
from concourse import bacc, mybir  # noqa: E402
from concourse.bass_utils import run_bass_kernel_spmd  # noqa: E402

LAST_RESULTS = None  # BassKernelResults of the most recent kernel() call

B, N, C, H = 4, 1024, 256, 8
HD = C // H           # 32
NQ = N // 2           # 512 queries per core
NKC = N // 128        # 8 key chunks
NQC = NQ // 128       # 4 query chunks
SCALE = HD ** -0.5
DELTA = 50.0          # pos bias shift: expbh = exp(bh - bmax + DELTA)
FP = mybir.dt.float32
BF = mybir.dt.bfloat16
AF = mybir.ActivationFunctionType
ALU = mybir.AluOpType
AX = mybir.AxisListType


def _build(w3, gh):
    """w3: Wpos[:,3] (8 floats); gh: sigmoid(gating) (8 floats) as immediates."""
    nc = bacc.Bacc("TRN2", target_bir_lowering=False)

    # bf16 inputs arrive as two [128, BLOBW] blobs (one DMA each):
    # [xT (1024) | xTq (512) | wq (256) | wk (256) | wp (256) | id128 (128)]
    BLOBW = N + NQ + 3 * C + 128
    d_blob = [nc.dram_tensor(f"blob{i}", (128, BLOBW), BF, kind="ExternalInput")
              for i in range(2)]
    d_va = nc.dram_tensor("va", (128, NKC * 272), BF, kind="ExternalInput")
    d_vp = nc.dram_tensor("vp", (128, NKC * 272), BF, kind="ExternalInput")
    # hi/lo-split Gram operands (bf16, fp32-accurate): rows 0-2 -2c_hi /
    # c_hi, 3-5 -2c_hi / c_lo, 6-8 -2c_lo / c_hi, 9-10 ones / sq_hi,sq_lo
    d_cT = nc.dram_tensor("cTb", (11, N), BF, kind="ExternalInput")
    d_cm2 = nc.dram_tensor("cm2b", (11, NQ), BF, kind="ExternalInput")
    d_sqq = nc.dram_tensor("sqq4", (128, NQC), FP, kind="ExternalInput")
    d_bp = nc.dram_tensor("bproj", (128, 2), FP, kind="ExternalInput")
    d_y = nc.dram_tensor("yT", (C, NQ), FP, kind="ExternalOutput")

    with tile.TileContext(nc) as tc:
        with (
            tc.tile_pool(name="const", bufs=1) as cpool,
            tc.tile_pool(name="work", bufs=3) as wpool,
            tc.tile_pool(name="ebuf", bufs=4) as epool,
            tc.tile_pool(name="etbuf", bufs=2) as etpool,
            tc.tile_pool(name="psw", bufs=2, space=bass.MemorySpace.PSUM) as pp,
            tc.tile_pool(name="psacc", bufs=2, space=bass.MemorySpace.PSUM) as pacc,
            tc.tile_pool(name="psT", bufs=2, space=bass.MemorySpace.PSUM) as pT,
        ):
            # ---- constants / inputs ----
            # small fp32 tensors first (pos prep depends only on these),
            # issued on the scalar queue so they don't serialize behind the
            # big bf16 blobs on sync.
            cT = cpool.tile([11, N], BF, tag="cT")
            cm2 = cpool.tile([11, NQ], BF, tag="cm2")
            sqq4 = cpool.tile([128, NQC], FP, tag="sqq4")
            bp2 = cpool.tile([128, 2], FP, tag="bp2")
            nc.scalar.dma_start(cT[:], d_cT[:])
            nc.scalar.dma_start(cm2[:], d_cm2[:])
            nc.scalar.dma_start(sqq4[:], d_sqq[:])
            nc.scalar.dma_start(bp2[:], d_bp[:])
            blob = [cpool.tile([128, BLOBW], BF, tag=f"blob{i}", name=f"blob{i}")
                    for i in range(2)]
            nc.sync.dma_start(blob[0][:], d_blob[0][:])
            nc.sync.dma_start(blob[1][:], d_blob[1][:])
            va = cpool.tile([128, NKC * 272], BF, tag="va")
            vp = cpool.tile([128, NKC * 272], BF, tag="vp")
            nc.sync.dma_start(va[:], d_va[:])
            nc.sync.dma_start(vp[:], d_vp[:])
            o_xT, o_xTq, o_wq, o_wk, o_wp, o_id = (
                0, N, N + NQ, N + NQ + C, N + NQ + 2 * C, N + NQ + 3 * C)
            xT = [blob[i][:, o_xT:o_xT + N] for i in range(2)]
            xTq = [blob[i][:, o_xTq:o_xTq + NQ] for i in range(2)]
            wq = [blob[i][:, o_wq:o_wq + C] for i in range(2)]
            wk = [blob[i][:, o_wk:o_wk + C] for i in range(2)]
            wp = [blob[i][:, o_wp:o_wp + C] for i in range(2)]
            id128 = blob[0][:, o_id:o_id + 128]
            sqq = [sqq4[:, qi:qi + 1] for qi in range(NQC)]
            bp = [bp2[:, i:i + 1] for i in range(2)]

            # ---- projections qT [C, NQ], kT [C, N] (bf16) ----
            # rows 96:128 (heads 3, 7) are mirrored to base-0 tiles: matmul
            # operands at base_partition 96 are not supported.
            q_sb = [cpool.tile([128, NQ], BF, tag=f"q{i}", name=f"qsb{i}") for i in range(2)]
            k_sb = [cpool.tile([128, N], BF, tag=f"k{i}", name=f"ksb{i}") for i in range(2)]
            q_e = [cpool.tile([32, NQ], BF, tag=f"qe{i}", name=f"qe{i}") for i in range(2)]
            k_e = [cpool.tile([32, N], BF, tag=f"ke{i}", name=f"ke{i}") for i in range(2)]
            for co in range(2):
                cs = slice(co * 128, (co + 1) * 128)
                ps = pp.tile([128, 1024], FP, tag="pw", name=f"qps{co}")
                for ci in range(2):
                    nc.tensor.matmul(ps[:, 0:NQ], wq[ci][:, cs], xTq[ci][:],
                                     start=(ci == 0), stop=(ci == 1))
                nc.vector.tensor_copy(q_sb[co][:], ps[:, 0:NQ])
                nc.vector.tensor_copy(q_e[co][:], ps[96:128, 0:NQ])
                ps2 = pp.tile([128, 1024], FP, tag="pw", name=f"kps{co}")
                for half in range(2):
                    hs = slice(half * 512, (half + 1) * 512)
                    for ci in range(2):
                        nc.tensor.matmul(ps2[:, hs], wk[ci][:, cs], xT[ci][:, hs],
                                         start=(ci == 0), stop=(ci == 1))
                nc.vector.tensor_copy(k_sb[co][:], ps2[:])
                nc.vector.tensor_copy(k_e[co][:], ps2[96:128, :])

            # ---- patch attention, key-major; o2 row 32 = S2 ----
            # o_sb tiles are 34 rows (row 33 zero from the padded V block) so
            # the combine transposes write 4-byte-aligned bf16 PSUM blocks.
            o2_sb = [cpool.tile([34, NQ], BF, tag=f"o2_{h}", name=f"o2sb{h}") for h in range(H)]

            def emit_patch(h):
                hc, j3 = h // 4, h % 4
                hr = j3 * 32
                q_h = q_e[hc][:] if j3 == 3 else q_sb[hc][hr:hr + 32, :]
                o2 = pacc.tile([34, NQ], FP, tag="acc", name=f"o2acc{h}")
                for kp in range(NKC // 2):
                    s2 = pp.tile([128, 1024], FP, tag="pw", name=f"s2_{h}_{kp}")
                    e2 = wpool.tile([128, 1024], BF, tag="e2", name=f"e2_{h}_{kp}")
                    for u in range(2):
                        kc = 2 * kp + u
                        ks = slice(kc * 128, (kc + 1) * 128)
                        k_h = k_e[hc][:, ks] if j3 == 3 else k_sb[hc][hr:hr + 32, ks]
                        nc.tensor.matmul(s2[:, u * NQ:(u + 1) * NQ], k_h, q_h,
                                         start=True, stop=True)
                    nc.scalar.activation(e2[:], s2[:], AF.Exp, scale=SCALE)
                    for u in range(2):
                        kc = 2 * kp + u
                        nc.tensor.matmul(o2[:], va[:, kc * 272 + h * 34:kc * 272 + (h + 1) * 34],
                                         e2[:, u * NQ:(u + 1) * NQ],
                                         start=(kc == 0), stop=(kc == NKC - 1))
                nc.vector.tensor_copy(o2_sb[h][:], o2[:])

            # ---- pos distance prep: d = sqrt(max(0, cm2.cT + sq_q)) ----
            # cT row 3 carries sq_k and cm2 row 3 is 1.0, so a single fp32
            # matmul yields -2c.c + sq_k; sq_q rides the DVE op as a
            # per-partition scalar, fused with the >=0 clamp.
            d_q = [cpool.tile([128, N], FP, tag=f"d{qi}", name=f"d{qi}") for qi in range(NQC)]
            dmax = [cpool.tile([128, 1], FP, tag=f"dmax{qi}", name=f"dmax{qi}") for qi in range(NQC)]
            dmin = [cpool.tile([128, 1], FP, tag=f"dmin{qi}", name=f"dmin{qi}") for qi in range(NQC)]

            def emit_prep(qi):
                qs = slice(qi * 128, (qi + 1) * 128)
                dps = pp.tile([128, 1024], FP, tag="pw", name=f"dps{qi}")
                for half in range(2):
                    hs = slice(half * 512, (half + 1) * 512)
                    nc.tensor.matmul(dps[:, hs], cm2[:, qs], cT[:, hs],
                                     start=True, stop=True)
                d2 = wpool.tile([128, 1024], FP, tag="d2", name=f"d2_{qi}")
                nc.vector.tensor_scalar(d2[:], dps[:], sqq[qi], 0.0,
                                        op0=ALU.add, op1=ALU.max)
                nc.scalar.sqrt(d_q[qi][:], d2[:])
                nc.vector.tensor_reduce(dmax[qi][:], d_q[qi][:], AX.X, ALU.max)
                nc.vector.tensor_reduce(dmin[qi][:], d_q[qi][:], AX.X, ALU.min)

            # emission order keeps the PE dense from the start (HAM warm):
            # two patch heads, then the pos prep (whose sqrts sit between
            # exp blocks on the scalar queue: 2 act-table switches total),
            # then the remaining patch heads.
            emit_patch(0)
            emit_patch(1)
            for qi in range(NQC):
                emit_prep(qi)
            for h in range(2, H):
                emit_patch(h)

            # ---- pos attention: e1 query-major -> DMA xbar transpose ----
            o1_sb = [cpool.tile([34, NQ], BF, tag=f"o1_{h}", name=f"o1sb{h}") for h in range(H)]
            for h in range(H):
                dext = dmax if w3[h] > 0 else dmin
                # e1T[p, j, qi, q] = e1[qi*128+q, 8p+j]
                e1T = etpool.tile([128, NKC, NQC, 128], BF, tag="e1T")
                for qi in range(NQC):
                    negm = wpool.tile([128, 1], FP, tag="negm")
                    nc.vector.tensor_scalar_mul(negm[:], dext[qi][:], -float(w3[h]))
                    e1 = epool.tile([128, N], BF, tag="e1")
                    nc.scalar.activation(e1[:], d_q[qi][:], AF.Exp,
                                         bias=negm[:], scale=float(w3[h]))
                    nc.sync.dma_start(e1T[:, :, qi, :], e1[:], transpose=True)
                o1 = pacc.tile([34, NQ], FP, tag="acc")
                for j in range(NKC):
                    nc.tensor.matmul(o1[:], vp[:, j * 272 + h * 34:j * 272 + (h + 1) * 34],
                                     e1T[:, j, :, :], start=(j == 0), stop=(j == NKC - 1))
                nc.vector.tensor_copy(o1_sb[h][:], o1[:])

            # ---- combine per query chunk: transpose accs, normalize, add ----
            oT = [cpool.tile([128, NQ], BF, tag=f"oT{i}", name=f"oTsb{i}") for i in range(2)]
            for qi in range(NQC):
                qs = slice(qi * 128, (qi + 1) * 128)
                T2 = pT.tile([128, 272], BF, tag="T")
                T1 = pT.tile([128, 272], BF, tag="T")
                for h in range(H):
                    nc.tensor.transpose(T2[:, h * 34:(h + 1) * 34],
                                        o2_sb[h][:, qs], id128[0:34, 0:34])
                    nc.tensor.transpose(T1[:, h * 34:(h + 1) * 34],
                                        o1_sb[h][:, qs], id128[0:34, 0:34])
                r2 = wpool.tile([128, H], FP, tag="r2")
                r1 = wpool.tile([128, H], FP, tag="r1")
                nc.vector.reciprocal(r2[:], T2[:, 32::34])
                nc.vector.reciprocal(r1[:], T1[:, 32::34])
                oq = wpool.tile([128, C], BF, tag="oq")
                for h in range(H):
                    t = wpool.tile([128, HD], FP, tag="t")
                    nc.vector.tensor_scalar_mul(t[:], T2[:, h * 34:h * 34 + 32],
                                                r2[:, h:h + 1])
                    nc.vector.scalar_tensor_tensor(
                        oq[:, h * 32:(h + 1) * 32], T1[:, h * 34:h * 34 + 32],
                        r1[:, h:h + 1], t[:], op0=ALU.mult, op1=ALU.add)
                for ci in range(2):
                    otp = pT.tile([128, 272], BF, tag="T", name=f"otp{qi}_{ci}")
                    nc.tensor.transpose(otp[:, 0:128], oq[:, ci * 128:(ci + 1) * 128],
                                        id128[:, :])
                    nc.vector.tensor_copy(oT[ci][:, qs], otp[:, 0:128])

            # ---- final projection yT = Wproj @ OT + bproj ----
            for co in range(2):
                cs = slice(co * 128, (co + 1) * 128)
                yp0 = pp.tile([128, 1024], FP, tag="pw", name=f"yp{co}")
                yp = yp0[:, 0:NQ]
                for ci in range(2):
                    nc.tensor.matmul(yp[:], wp[ci][:, cs], oT[ci][:],
                                     start=(ci == 0), stop=(ci == 1))
                y = wpool.tile([128, NQ], FP, tag="y")
                nc.scalar.activation(y[:], yp[:], AF.Identity, bias=bp[co][:])
                nc.sync.dma_start(d_y[cs, :], y[:])

    nc.compile()
    return nc


def kernel(x, voxel_coord, Wqk, Wv, Wpos, bpos, Wproj, bproj, gating):
    x = np.asarray(x, np.float32)
    c = np.asarray(voxel_coord, np.float32)
    Wqk = np.asarray(Wqk, np.float32)
    Wv = np.asarray(Wv, np.float32)
    Wpos = np.asarray(Wpos, np.float32)
    bpos = np.asarray(bpos, np.float32)
    Wproj = np.asarray(Wproj, np.float32)
    bproj = np.asarray(bproj, np.float32)
    gating = np.asarray(gating, np.float32)

    import ml_dtypes
    bf16 = ml_dtypes.bfloat16

    w3 = [float(v) for v in Wpos[:, 3]]
    gh = [float(v) for v in 1.0 / (1.0 + np.exp(-gating))]
    nc = _build(w3, gh)

    WqT = np.ascontiguousarray(Wqk[:C].T).astype(bf16)
    WkT = np.ascontiguousarray(Wqk[C:].T).astype(bf16)
    WprojT = np.ascontiguousarray(Wproj.T).astype(bf16)
    bp2 = np.ascontiguousarray(bproj.reshape(2, 128).T)
    id128 = np.eye(128, dtype=np.float32)

    # v = x @ Wv.T; Wv is identity in this model, skip the matmul then.
    if np.array_equal(Wv, np.eye(C, dtype=np.float32)):
        v_full = x
    else:
        v_full = x @ Wv.T

    c = c - c.mean(axis=1, keepdims=True)  # shrink |c|^2 for Gram precision

    in_maps = []
    for core in range(8):
        b, r = core // 2, core % 2
        qs = slice(r * NQ, (r + 1) * NQ)
        xTb = np.ascontiguousarray(x[b].T)                      # (C, N) f32
        sq = np.sum(c[b] * c[b], axis=1).astype(np.float32)     # (N,)
        # hi/lo bf16 split of the Gram operands: the matmul computes
        # -2(hi.hi' + hi.lo' + lo.hi') + sq_hi + sq_lo in one bf16 pass
        # (the dropped -2 lo.lo' term is <= ~0.02).
        ct = c[b].T                                             # (3, N)
        ct_hi = ct.astype(bf16).astype(np.float32)
        ct_lo = (ct - ct_hi).astype(bf16).astype(np.float32)
        sq_hi = sq.astype(bf16).astype(np.float32)
        sq_lo = (sq - sq_hi).astype(bf16).astype(np.float32)
        cTb = np.empty((11, N), np.float32)
        cTb[0:3] = ct_hi
        cTb[3:6] = ct_lo
        cTb[6:9] = ct_hi
        cTb[9] = sq_hi
        cTb[10] = sq_lo
        cm2 = np.empty((11, NQ), np.float32)
        cm2[0:3] = -2.0 * ct_hi[:, qs]
        cm2[3:6] = -2.0 * ct_hi[:, qs]
        cm2[6:9] = -2.0 * ct_lo[:, qs]
        cm2[9] = 1.0
        cm2[10] = 1.0
        bh = -(Wpos[:, :3] @ c[b].T) + bpos[:, None]            # (H, N)
        expbh = np.exp(bh - bh.max(axis=1, keepdims=True) + DELTA)  # (H, N)

        # patch V block: [(1-g)*v_h | ones] per (kc, h), native key order
        va = np.zeros((128, NKC * 272), np.float32)
        # pos V block: [g*expbh*v_h | expbh], interleaved key order k=8p+j
        vp = np.zeros((128, NKC * 272), np.float32)
        vb = v_full[b]                                          # (N, C)
        for kc in range(NKC):
            keys = np.arange(kc * 128, (kc + 1) * 128)
            for h in range(H):
                col = kc * 272 + h * 34
                va[:, col:col + 32] = (1.0 - gh[h]) * vb[keys, h * 32:(h + 1) * 32]
                va[:, col + 32] = 1.0
                vp[:, col:col + 32] = (gh[h] * expbh[h, keys, None]
                                       * vb[keys, h * 32:(h + 1) * 32])
                vp[:, col + 32] = expbh[h, keys]

        blobs = []
        for i in range(2):
            rs = slice(i * 128, (i + 1) * 128)
            idpart = id128 if i == 0 else np.zeros((128, 128), np.float32)
            blobs.append(np.concatenate([
                xTb[rs].astype(bf16), xTb[rs, qs].astype(bf16),
                WqT[rs], WkT[rs], WprojT[rs], idpart.astype(bf16),
            ], axis=1))
        in_maps.append({
            "blob0": np.ascontiguousarray(blobs[0]),
            "blob1": np.ascontiguousarray(blobs[1]),
            "va": va.astype(bf16), "vp": vp.astype(bf16),
            "cTb": cTb.astype(bf16), "cm2b": cm2.astype(bf16),
            "sqq4": np.ascontiguousarray(sq[qs].reshape(NQC, 128).T),
            "bproj": bp2,
        })

    global LAST_RESULTS
    LAST_RESULTS = run_bass_kernel_spmd(nc, in_maps, list(range(8)))
    res = LAST_RESULTS.results
    out = np.empty((B, N, C), np.float32)
    for core in range(8):
        b, r = core // 2, core % 2
        out[b, r * NQ:(r + 1) * NQ, :] = res[core]["yT"].T
    return out


# revision 26
# speedup vs baseline: 1.3430x; 1.0205x over previous
"""GPSA (gated positional self-attention) Bass kernel for Trainium2, v2.

Sharding: 8 cores = 4 batches x 2 query-halves. Each core: full keys
(N=1024), 512 queries, all 8 heads.

Per-head math (core = batch b, query half r):
  patch: e2 = exp(scale * k^T q) computed key-major [keys, q] (no max
         needed, logits are tiny). PV uses a 33-column V-block
         [(1-g)*v_h | ones] so column 32 of the accumulator carries S2.
  pos:   e1 = exp(w3_h * d - m1) query-major [q, keys]; m1 = w3*dmax or
         w3*dmin (exact row max of w3*d). The per-key bias term
         exp(bh - bmax + DELTA) is folded into the pos V-block host-side:
         [g*expbh*v_h | expbh], so column 32 carries U1 = sum e1*expbh.
         e1 is transposed to key-major via DMA xbar transpose (bf16,
         3D out AP -> keys land interleaved k=8p+j; vp rows are host-
         permuted to match).
  combine (per query chunk qi): PE-transpose both 33-row accumulators to
         [q, 33]; reciprocal of column 32 gives per-partition 1/S2, 1/U1;
         two DVE ops apply both normalizations and the add. Final
         transpose back to [C, q] for the output projection.

All matmuls bf16 except the distance Gram trick (fp32 for cancellation).
Single act-table switch: all sqrts emitted before all exps.
"""
import sys
import numpy as np

sys.path.insert(0, "/opt/trn_rl_repo")

import concourse.bass as bass  # noqa: E402
import concourse.tile as tile  # noqa: E402
from concourse import bacc, mybir  # noqa: E402
from concourse.bass_utils import run_bass_kernel_spmd  # noqa: E402

LAST_RESULTS = None  # BassKernelResults of the most recent kernel() call

B, N, C, H = 4, 1024, 256, 8
HD = C // H           # 32
NQ = N // 2           # 512 queries per core
NKC = N // 128        # 8 key chunks
NQC = NQ // 128       # 4 query chunks
SCALE = HD ** -0.5
DELTA = 50.0          # pos bias shift: expbh = exp(bh - bmax + DELTA)
FP = mybir.dt.float32
BF = mybir.dt.bfloat16
AF = mybir.ActivationFunctionType
ALU = mybir.AluOpType
AX = mybir.AxisListType


def _build(w3, gh):
    """w3: Wpos[:,3] (8 floats); gh: sigmoid(gating) (8 floats) as immediates."""
    nc = bacc.Bacc("TRN2", target_bir_lowering=False)

    # bf16 inputs arrive as two [128, BLOBW] blobs (one DMA each):
    # [xT (1024) | xTq (512) | wq (256) | wk (256) | wp (256) | id128 (128)]
    BLOBW = N + NQ + 3 * C + 128
    d_blob = [nc.dram_tensor(f"blob{i}", (128, BLOBW), BF, kind="ExternalInput")
              for i in range(2)]
    d_va = nc.dram_tensor("va", (128, NKC * 272), BF, kind="ExternalInput")
    d_vp = nc.dram_tensor("vp", (128, NKC * 272), BF, kind="ExternalInput")
    # hi/lo-split Gram operands (bf16, fp32-accurate): rows 0-2 -2c_hi /
    # c_hi, 3-5 -2c_hi / c_lo, 6-8 -2c_lo / c_hi, 9-10 ones / sq_hi,sq_lo
    d_cT = nc.dram_tensor("cTb", (11, N), BF, kind="ExternalInput")
    d_cm2 = nc.dram_tensor("cm2b", (11, NQ), BF, kind="ExternalInput")
    d_sqq = nc.dram_tensor("sqq4", (128, NQC), FP, kind="ExternalInput")
    d_bp = nc.dram_tensor("bproj", (128, 2), FP, kind="ExternalInput")
    d_y = nc.dram_tensor("yT", (C, NQ), FP, kind="ExternalOutput")

    with tile.TileContext(nc) as tc:
        with (
            tc.tile_pool(name="const", bufs=1) as cpool,
            tc.tile_pool(name="work", bufs=3) as wpool,
            tc.tile_pool(name="ebuf", bufs=4) as epool,
            tc.tile_pool(name="etbuf", bufs=2) as etpool,
            tc.tile_pool(name="psw", bufs=2, space=bass.MemorySpace.PSUM) as pp,
            tc.tile_pool(name="psacc", bufs=2, space=bass.MemorySpace.PSUM) as pacc,
            tc.tile_pool(name="psT", bufs=2, space=bass.MemorySpace.PSUM) as pT,
        ):
            # ---- constants / inputs ----
            # small fp32 tensors first (pos prep depends only on these),
            # issued on the scalar queue so they don't serialize behind the
            # big bf16 blobs on sync.
            cT = cpool.tile([11, N], BF, tag="cT")
            cm2 = cpool.tile([11, NQ], BF, tag="cm2")
            sqq4 = cpool.tile([128, NQC], FP, tag="sqq4")
            bp2 = cpool.tile([128, 2], FP, tag="bp2")
            nc.scalar.dma_start(cT[:], d_cT[:])
            nc.scalar.dma_start(cm2[:], d_cm2[:])
            nc.scalar.dma_start(sqq4[:], d_sqq[:])
            nc.scalar.dma_start(bp2[:], d_bp[:])
            blob = [cpool.tile([128, BLOBW], BF, tag=f"blob{i}", name=f"blob{i}")
                    for i in range(2)]
            nc.sync.dma_start(blob[0][:], d_blob[0][:])
            nc.sync.dma_start(blob[1][:], d_blob[1][:])
            va = cpool.tile([128, NKC * 272], BF, tag="va")
            vp = cpool.tile([128, NKC * 272], BF, tag="vp")
            nc.sync.dma_start(va[:], d_va[:])
            nc.sync.dma_start(vp[:], d_vp[:])
            o_xT, o_xTq, o_wq, o_wk, o_wp, o_id = (
                0, N, N + NQ, N + NQ + C, N + NQ + 2 * C, N + NQ + 3 * C)
            xT = [blob[i][:, o_xT:o_xT + N] for i in range(2)]
            xTq = [blob[i][:, o_xTq:o_xTq + NQ] for i in range(2)]
            wq = [blob[i][:, o_wq:o_wq + C] for i in range(2)]
            wk = [blob[i][:, o_wk:o_wk + C] for i in range(2)]
            wp = [blob[i][:, o_wp:o_wp + C] for i in range(2)]
            id128 = blob[0][:, o_id:o_id + 128]
            sqq = [sqq4[:, qi:qi + 1] for qi in range(NQC)]
            bp = [bp2[:, i:i + 1] for i in range(2)]

            # ---- PE HAM warm-up / keep-warm ----
            # The HAM clock gate only lifts to 8/8 after ~3.4us of *dense* PE
            # activity, and this kernel's phases leave the PE ~40-60% idle,
            # so it otherwise runs the whole kernel at 1.2 GHz. Dummy
            # matmuls (no data deps -> scheduled into idle slots) keep it at
            # 2.4 GHz. The warm pool reuses pw buffers, which are free
            # during the DMA prologue and the pos phase.
            wrm_a = cpool.tile([128, 128], BF, tag="wrma")
            wrm_b = cpool.tile([128, 512], BF, tag="wrmb")
            nc.vector.memset(wrm_a[:], 0.0)
            nc.vector.memset(wrm_b[:], 0.0)

            def emit_warm(n, name):
                w = pp.tile([128, 1024], FP, tag="pw", name=f"warm{name}")
                for u in range(n):
                    nc.tensor.matmul(w[:, 0:512], wrm_a[:], wrm_b[:],
                                     start=True, stop=True)

            emit_warm(12, "boot")

            # ---- projections qT [C, NQ], kT [C, N] (bf16) ----
            # rows 96:128 (heads 3, 7) are mirrored to base-0 tiles: matmul
            # operands at base_partition 96 are not supported.
            q_sb = [cpool.tile([128, NQ], BF, tag=f"q{i}", name=f"qsb{i}") for i in range(2)]
            k_sb = [cpool.tile([128, N], BF, tag=f"k{i}", name=f"ksb{i}") for i in range(2)]
            q_e = [cpool.tile([32, NQ], BF, tag=f"qe{i}", name=f"qe{i}") for i in range(2)]
            k_e = [cpool.tile([32, N], BF, tag=f"ke{i}", name=f"ke{i}") for i in range(2)]
            for co in range(2):
                cs = slice(co * 128, (co + 1) * 128)
                ps = pp.tile([128, 1024], FP, tag="pw", name=f"qps{co}")
                for ci in range(2):
                    nc.tensor.matmul(ps[:, 0:NQ], wq[ci][:, cs], xTq[ci][:],
                                     start=(ci == 0), stop=(ci == 1))
                nc.vector.tensor_copy(q_sb[co][:], ps[:, 0:NQ])
                nc.vector.tensor_copy(q_e[co][:], ps[96:128, 0:NQ])
                ps2 = pp.tile([128, 1024], FP, tag="pw", name=f"kps{co}")
                for half in range(2):
                    hs = slice(half * 512, (half + 1) * 512)
                    for ci in range(2):
                        nc.tensor.matmul(ps2[:, hs], wk[ci][:, cs], xT[ci][:, hs],
                                         start=(ci == 0), stop=(ci == 1))
                nc.vector.tensor_copy(k_sb[co][:], ps2[:])
                nc.vector.tensor_copy(k_e[co][:], ps2[96:128, :])

            # ---- patch attention, key-major; o2 row 32 = S2 ----
            # o_sb tiles are 34 rows (row 33 zero from the padded V block) so
            # the combine transposes write 4-byte-aligned bf16 PSUM blocks.
            o2_sb = [cpool.tile([34, NQ], BF, tag=f"o2_{h}", name=f"o2sb{h}") for h in range(H)]

            def emit_patch(h):
                hc, j3 = h // 4, h % 4
                hr = j3 * 32
                q_h = q_e[hc][:] if j3 == 3 else q_sb[hc][hr:hr + 32, :]
                o2 = pacc.tile([34, NQ], FP, tag="acc", name=f"o2acc{h}")
                for kp in range(NKC // 2):
                    s2 = pp.tile([128, 1024], FP, tag="pw", name=f"s2_{h}_{kp}")
                    e2 = wpool.tile([128, 1024], BF, tag="e2", name=f"e2_{h}_{kp}")
                    for u in range(2):
                        kc = 2 * kp + u
                        ks = slice(kc * 128, (kc + 1) * 128)
                        k_h = k_e[hc][:, ks] if j3 == 3 else k_sb[hc][hr:hr + 32, ks]
                        nc.tensor.matmul(s2[:, u * NQ:(u + 1) * NQ], k_h, q_h,
                                         start=True, stop=True)
                    nc.scalar.activation(e2[:], s2[:], AF.Exp, scale=SCALE)
                    for u in range(2):
                        kc = 2 * kp + u
                        nc.tensor.matmul(o2[:], va[:, kc * 272 + h * 34:kc * 272 + (h + 1) * 34],
                                         e2[:, u * NQ:(u + 1) * NQ],
                                         start=(kc == 0), stop=(kc == NKC - 1))
                nc.vector.tensor_copy(o2_sb[h][:], o2[:])

            # ---- pos distance prep: d = sqrt(max(0, cm2.cT + sq_q)) ----
            # cT row 3 carries sq_k and cm2 row 3 is 1.0, so a single fp32
            # matmul yields -2c.c + sq_k; sq_q rides the DVE op as a
            # per-partition scalar, fused with the >=0 clamp.
            d_q = [cpool.tile([128, N], FP, tag=f"d{qi}", name=f"d{qi}") for qi in range(NQC)]
            dmax = [cpool.tile([128, 1], FP, tag=f"dmax{qi}", name=f"dmax{qi}") for qi in range(NQC)]
            dmin = [cpool.tile([128, 1], FP, tag=f"dmin{qi}", name=f"dmin{qi}") for qi in range(NQC)]

            def emit_prep(qi):
                qs = slice(qi * 128, (qi + 1) * 128)
                dps = pp.tile([128, 1024], FP, tag="pw", name=f"dps{qi}")
                for half in range(2):
                    hs = slice(half * 512, (half + 1) * 512)
                    nc.tensor.matmul(dps[:, hs], cm2[:, qs], cT[:, hs],
                                     start=True, stop=True)
                d2 = wpool.tile([128, 1024], FP, tag="d2", name=f"d2_{qi}")
                nc.vector.tensor_scalar(d2[:], dps[:], sqq[qi], 0.0,
                                        op0=ALU.add, op1=ALU.max)
                nc.scalar.sqrt(d_q[qi][:], d2[:])
                nc.vector.tensor_reduce(dmax[qi][:], d_q[qi][:], AX.X, ALU.max)
                nc.vector.tensor_reduce(dmin[qi][:], d_q[qi][:], AX.X, ALU.min)

            # emission order keeps the PE dense from the start (HAM warm):
            # two patch heads, then the pos prep (whose sqrts sit between
            # exp blocks on the scalar queue: 2 act-table switches total),
            # then the remaining patch heads.
            emit_patch(0)
            emit_patch(1)
            for qi in range(NQC):
                emit_prep(qi)
            for h in range(2, H):
                emit_patch(h)

            # ---- pos attention: e1 query-major -> DMA xbar transpose ----
            o1_sb = [cpool.tile([34, NQ], BF, tag=f"o1_{h}", name=f"o1sb{h}") for h in range(H)]
            for h in range(H):
                dext = dmax if w3[h] > 0 else dmin
                # e1T[p, j, qi, q] = e1[qi*128+q, 8p+j]
                e1T = etpool.tile([128, NKC, NQC, 128], BF, tag="e1T")
                for qi in range(NQC):
                    negm = wpool.tile([128, 1], FP, tag="negm")
                    nc.vector.tensor_scalar_mul(negm[:], dext[qi][:], -float(w3[h]))
                    e1 = epool.tile([128, N], BF, tag="e1")
                    nc.scalar.activation(e1[:], d_q[qi][:], AF.Exp,
                                         bias=negm[:], scale=float(w3[h]))
                    nc.sync.dma_start(e1T[:, :, qi, :], e1[:], transpose=True)
                o1 = pacc.tile([34, NQ], FP, tag="acc")
                for j in range(NKC):
                    nc.tensor.matmul(o1[:], vp[:, j * 272 + h * 34:j * 272 + (h + 1) * 34],
                                     e1T[:, j, :, :], start=(j == 0), stop=(j == NKC - 1))
                nc.vector.tensor_copy(o1_sb[h][:], o1[:])
                emit_warm(6, f"pos{h}")

            # ---- combine per query chunk: transpose accs, normalize, add ----
            oT = [cpool.tile([128, NQ], BF, tag=f"oT{i}", name=f"oTsb{i}") for i in range(2)]
            for qi in range(NQC):
                qs = slice(qi * 128, (qi + 1) * 128)
                T2 = pT.tile([128, 272], BF, tag="T")
                T1 = pT.tile([128, 272], BF, tag="T")
                for h in range(H):
                    nc.tensor.transpose(T2[:, h * 34:(h + 1) * 34],
                                        o2_sb[h][:, qs], id128[0:34, 0:34])
                    nc.tensor.transpose(T1[:, h * 34:(h + 1) * 34],
                                        o1_sb[h][:, qs], id128[0:34, 0:34])
                r2 = wpool.tile([128, H], FP, tag="r2")
                r1 = wpool.tile([128, H], FP, tag="r1")
                nc.vector.reciprocal(r2[:], T2[:, 32::34])
                nc.vector.reciprocal(r1[:], T1[:, 32::34])
                oq = wpool.tile([128, C], BF, tag="oq")
                for h in range(H):
                    t = wpool.tile([128, HD], FP, tag="t")
                    nc.vector.tensor_scalar_mul(t[:], T2[:, h * 34:h * 34 + 32],
                                                r2[:, h:h + 1])
                    nc.vector.scalar_tensor_tensor(
                        oq[:, h * 32:(h + 1) * 32], T1[:, h * 34:h * 34 + 32],
                        r1[:, h:h + 1], t[:], op0=ALU.mult, op1=ALU.add)
                for ci in range(2):
                    otp = pT.tile([128, 272], BF, tag="T", name=f"otp{qi}_{ci}")
                    nc.tensor.transpose(otp[:, 0:128], oq[:, ci * 128:(ci + 1) * 128],
                                        id128[:, :])
                    nc.vector.tensor_copy(oT[ci][:, qs], otp[:, 0:128])

            # ---- final projection yT = Wproj @ OT + bproj ----
            for co in range(2):
                cs = slice(co * 128, (co + 1) * 128)
                yp0 = pp.tile([128, 1024], FP, tag="pw", name=f"yp{co}")
                yp = yp0[:, 0:NQ]
                for ci in range(2):
                    nc.tensor.matmul(yp[:], wp[ci][:, cs], oT[ci][:],
                                     start=(ci == 0), stop=(ci == 1))
                y = wpool.tile([128, NQ], FP, tag="y")
                nc.scalar.activation(y[:], yp[:], AF.Identity, bias=bp[co][:])
                nc.sync.dma_start(d_y[cs, :], y[:])

    nc.compile()
    return nc


def kernel(x, voxel_coord, Wqk, Wv, Wpos, bpos, Wproj, bproj, gating):
    x = np.asarray(x, np.float32)
    c = np.asarray(voxel_coord, np.float32)
    Wqk = np.asarray(Wqk, np.float32)
    Wv = np.asarray(Wv, np.float32)
    Wpos = np.asarray(Wpos, np.float32)
    bpos = np.asarray(bpos, np.float32)
    Wproj = np.asarray(Wproj, np.float32)
    bproj = np.asarray(bproj, np.float32)
    gating = np.asarray(gating, np.float32)

    import ml_dtypes
    bf16 = ml_dtypes.bfloat16

    w3 = [float(v) for v in Wpos[:, 3]]
    gh = [float(v) for v in 1.0 / (1.0 + np.exp(-gating))]
    nc = _build(w3, gh)

    WqT = np.ascontiguousarray(Wqk[:C].T).astype(bf16)
    WkT = np.ascontiguousarray(Wqk[C:].T).astype(bf16)
    WprojT = np.ascontiguousarray(Wproj.T).astype(bf16)
    bp2 = np.ascontiguousarray(bproj.reshape(2, 128).T)
    id128 = np.eye(128, dtype=np.float32)

    # v = x @ Wv.T; Wv is identity in this model, skip the matmul then.
    if np.array_equal(Wv, np.eye(C, dtype=np.float32)):
        v_full = x
    else:
        v_full = x @ Wv.T

    c = c - c.mean(axis=1, keepdims=True)  # shrink |c|^2 for Gram precision

    in_maps = []
    for core in range(8):
        b, r = core // 2, core % 2
        qs = slice(r * NQ, (r + 1) * NQ)
        xTb = np.ascontiguousarray(x[b].T)                      # (C, N) f32
        sq = np.sum(c[b] * c[b], axis=1).astype(np.float32)     # (N,)
        # hi/lo bf16 split of the Gram operands: the matmul computes
        # -2(hi.hi' + hi.lo' + lo.hi') + sq_hi + sq_lo in one bf16 pass
        # (the dropped -2 lo.lo' term is <= ~0.02).
        ct = c[b].T                                             # (3, N)
        ct_hi = ct.astype(bf16).astype(np.float32)
        ct_lo = (ct - ct_hi).astype(bf16).astype(np.float32)
        sq_hi = sq.astype(bf16).astype(np.float32)
        sq_lo = (sq - sq_hi).astype(bf16).astype(np.float32)
        cTb = np.empty((11, N), np.float32)
        cTb[0:3] = ct_hi
        cTb[3:6] = ct_lo
        cTb[6:9] = ct_hi
        cTb[9] = sq_hi
        cTb[10] = sq_lo
        cm2 = np.empty((11, NQ), np.float32)
        cm2[0:3] = -2.0 * ct_hi[:, qs]
        cm2[3:6] = -2.0 * ct_hi[:, qs]
        cm2[6:9] = -2.0 * ct_lo[:, qs]
        cm2[9] = 1.0
        cm2[10] = 1.0
        bh = -(Wpos[:, :3] @ c[b].T) + bpos[:, None]            # (H, N)
        expbh = np.exp(bh - bh.max(axis=1, keepdims=True) + DELTA)  # (H, N)

        # patch V block: [(1-g)*v_h | ones] per (kc, h), native key order
        va = np.zeros((128, NKC * 272), np.float32)
        # pos V block: [g*expbh*v_h | expbh], interleaved key order k=8p+j
        vp = np.zeros((128, NKC * 272), np.float32)
        vb = v_full[b]                                          # (N, C)
        for kc in range(NKC):
            keys = np.arange(kc * 128, (kc + 1) * 128)
            for h in range(H):
                col = kc * 272 + h * 34
                va[:, col:col + 32] = (1.0 - gh[h]) * vb[keys, h * 32:(h + 1) * 32]
                va[:, col + 32] = 1.0
                vp[:, col:col + 32] = (gh[h] * expbh[h, keys, None]
                                       * vb[keys, h * 32:(h + 1) * 32])
                vp[:, col + 32] = expbh[h, keys]

        blobs = []
        for i in range(2):
            rs = slice(i * 128, (i + 1) * 128)
            idpart = id128 if i == 0 else np.zeros((128, 128), np.float32)
            blobs.append(np.concatenate([
                xTb[rs].astype(bf16), xTb[rs, qs].astype(bf16),
                WqT[rs], WkT[rs], WprojT[rs], idpart.astype(bf16),
            ], axis=1))
        in_maps.append({
            "blob0": np.ascontiguousarray(blobs[0]),
            "blob1": np.ascontiguousarray(blobs[1]),
            "va": va.astype(bf16), "vp": vp.astype(bf16),
            "cTb": cTb.astype(bf16), "cm2b": cm2.astype(bf16),
            "sqq4": np.ascontiguousarray(sq[qs].reshape(NQC, 128).T),
            "bproj": bp2,
        })

    global LAST_RESULTS
    LAST_RESULTS = run_bass_kernel_spmd(nc, in_maps, list(range(8)))
    res = LAST_RESULTS.results
    out = np.empty((B, N, C), np.float32)
    for core in range(8):
        b, r = core // 2, core % 2
        out[b, r * NQ:(r + 1) * NQ, :] = res[core]["yT"].T
    return out
